# revision 30
# baseline (speedup 1.0000x reference)
"""CRF loss kernel for Trainium2 (8 NeuronCores).

Strategy
--------
The CRF forward scan  Z_{t+1} = logsumexp_i(Z_t[:,i] + Tr[i,j]) + logit_{t+1}
is rewritten in exp-domain as a *normalized* plain matmul recurrence:

    a_{t+1} = (a_t @ exp(Tr)) * G_{t+1},   G = exp(logits)  (precomputed)
    U_t     = sum_j a_t[b, j]                               (stored per step)

which runs on the TensorEngine: per step, 64 matmuls of
[K=128]x[M=128]x[N=32] with the exp(transition) tiles as stationary
operands (fp8, fast-weight-load) and the state a [i, b] as the moving
operand (bf16).  Every R steps the state is renormalized by 1/U_{t-1}
(one step stale, so the reciprocal/broadcast are off the critical
path).  All logarithms are LAZY: U_t is stored into a flat SBUF buffer
each step; a single end-of-kernel Ln + host-precomputed mask (encoding
the t == len[b]-1 harvest and the renorm compensation, both computable
on the host from `lengths`) reduces it to log Z[b].  The Scalar
engine's activation table holds Exp for the whole kernel (the naive
per-step Exp/Ln alternation costs two ~1.3us table reloads per step).

Per scan step the PE stream is software-pipelined:  phase-1 matmuls
(k = 0..3) of step t+1 only need early state-quarters of step t, so
they overlap the cross-engine latency of the late quarters; the state
multiply is issued as four jc-pair quarters, each in its own PSUM bank
(avoids PE-write/DVE-read bank serialization).

The projection G = exp(enc @ W.T + b) feeds the scan from DRAM in 4
t-chunks; its blocks are interleaved into the early scan steps so PE
never drains.  The gold-path score gathers exp-domain values straight
from G (and transition rows) via indirect DMA into two [128, 64, 32]
buffers; one bulk Ln + one fused multiply-reduce each at the end fold
them to per-batch scores (the bias term of the emit score is folded in
on the host).

The scan is inherently sequential over T and too small in B to shard
without per-step cross-core communication (collective latency >> step
time), so each core runs the identical full problem (data-parallel
replication costs nothing in wall time) and the host takes core 0's
scalar output.
"""
import sys
import os

sys.path.insert(0, "/opt/trn_rl_repo")

import numpy as np
import ml_dtypes

import concourse.bass as bass
import concourse.bacc as bacc
import concourse.tile as tile
from concourse import mybir
from concourse.bass_utils import run_bass_kernel_spmd

T, B, H, V = 256, 32, 512, 1024
P = 128
JC = V // P          # 8 vocab chunks
KH = H // P          # 4 hidden chunks
NTB = (T * B) // 512  # 16 tb-chunks of 512 for the projection
TB = T * B
R = 8                # renormalize the scan state every R steps
N_CORES = 8
NCHUNK = 4           # gexp DRAM tensor t-chunk split
TCH = T // NCHUNK    # 64 steps per chunk

F32 = mybir.dt.float32
BF16 = mybir.dt.bfloat16
FP8 = mybir.dt.float8e4

EHAT_DT = FP8        # dtype of exp(transition) stationary tiles
SCAN_STEPS = T       # full scan


def _renorm_steps(steps=SCAN_STEPS, r=R):
    return [t for t in range(steps) if t % r == r - 1 and t != steps - 1]


def _build_program(steps=SCAN_STEPS, ehat_dt=EHAT_DT, skip_gather=False,
                   skip_renorm=False):
    nc = bacc.Bacc("TRN2", target_bir_lowering=False, debug=False,
                   enable_asserts=False, num_devices=N_CORES)

    encT_d = nc.dram_tensor("encT", [H, TB], BF16, kind="ExternalInput")
    wt_d = nc.dram_tensor("Wt", [H, V], F32, kind="ExternalInput")
    bcol_d = nc.dram_tensor("bcol", [P, JC], F32, kind="ExternalInput")
    trans_d = nc.dram_tensor("trans", [V, V], F32, kind="ExternalInput")
    selM_d = nc.dram_tensor("selM", [P, TB // P], F32, kind="ExternalInput")
    eoff_d = nc.dram_tensor("eoff", [P, 64], mybir.dt.int32, kind="ExternalInput")
    esel_d = nc.dram_tensor("esel", [P, 64, B], F32, kind="ExternalInput")
    toff_d = nc.dram_tensor("toff", [P, 64], mybir.dt.int32, kind="ExternalInput")
    tsel_d = nc.dram_tensor("tsel", [P, 64, B], F32, kind="ExternalInput")
    foldE_d = nc.dram_tensor("foldE", [P, B], F32, kind="ExternalInput")
    bterm_d = nc.dram_tensor("bterm", [1, B], F32, kind="ExternalInput")
    loss_d = nc.dram_tensor("loss", [1, 1], F32, kind="ExternalOutput")

    renorms = set(_renorm_steps(steps))
    CW = TB // P  # 64 columns in the reshaped U / selM layout

    with tile.TileContext(nc) as tc:
        with tc.tile_pool(name="const", bufs=1) as cp, \
             tc.tile_pool(name="dram", bufs=1, space="DRAM") as dp:
            gexp_c = []
            for c in range(NCHUNK):
                gx = dp.tile([V, TCH, B], BF16, tag=f"gexp{c}", name=f"gexp{c}")
                gexp_c.append(gx)
            uscr = dp.tile([1, TB], F32, tag="uscr")

            # ---------------- phase A: constants -------------------------
            ehat = cp.tile([P, JC, V], ehat_dt, tag="ehat")
            wt_sb = cp.tile([P, KH, V], BF16, tag="wt")
            ones_c = cp.tile([P, 1], BF16, tag="ones_c")
            ones_r = cp.tile([1, P], F32, tag="ones_r")
            ones_f = cp.tile([P, 1], F32, tag="ones_f")
            bcol_sb = cp.tile([P, JC], F32, tag="bcol")
            selM_sb = cp.tile([P, CW], F32, tag="selM")
            Uall = cp.tile([1, TB], F32, tag="Uall")
            eoff_sb = cp.tile([P, 64], mybir.dt.int32, tag="eoff")
            esel_sb = cp.tile([P, 64, B], F32, tag="esel")
            toff_sb = cp.tile([P, 64], mybir.dt.int32, tag="toff")
            tsel_sb = cp.tile([P, 64, B], F32, tag="tsel")
            foldE_sb = cp.tile([P, B], F32, tag="foldE")
            bterm_sb = cp.tile([1, B], F32, tag="bterm")
            egall = cp.tile([P, 64, B], BF16, tag="egall")
            tgall = cp.tile([P, 64, B], F32, tag="tgall")

            nc.vector.memset(ones_c[:], 1.0)
            nc.vector.memset(ones_r[:], 1.0)
            nc.vector.memset(ones_f[:], 1.0)
            nc.sync.dma_start(bcol_sb[:], bcol_d.ap()[:])
            nc.sync.dma_start(toff_sb[:], toff_d.ap()[:])
            nc.sync.dma_start(eoff_sb[:], eoff_d.ap()[:])

            with tc.tile_pool(name="stage", bufs=4) as stp, \
                 tc.tile_pool(name="proj_ps", bufs=1, space="PSUM") as pps, \
                 tc.tile_pool(name="enc_p", bufs=2) as encp, \
                 tc.tile_pool(name="ge_p", bufs=3) as gep, \
                 tc.tile_pool(name="scan_sb", bufs=3) as ssb, \
                 tc.tile_pool(name="lt_p", bufs=3) as ltp, \
                 tc.tile_pool(name="s_ps0", bufs=1, space="PSUM") as sps0, \
                 tc.tile_pool(name="s_ps1", bufs=1, space="PSUM") as sps1, \
                 tc.tile_pool(name="s_ps2", bufs=1, space="PSUM") as sps2, \
                 tc.tile_pool(name="s_ps3", bufs=1, space="PSUM") as sps3, \
                 tc.tile_pool(name="u_ps", bufs=1, space="PSUM") as ups, \
                 tc.tile_pool(name="fold_ps", bufs=1, space="PSUM") as fps, \
                 tc.tile_pool(name="bc_ps", bufs=1, space="PSUM") as bps, \
                 tc.tile_pool(name="sm", bufs=4) as smp, \
                 tc.tile_pool(name="fin", bufs=1) as fin:

                # single-slot U bank: rewriting the same [1, B] slice each
                # step makes the ACT copy a tracked WAR predecessor of the
                # next step's start (which zeroes the whole 2KB region).
                u1 = ups.tile([1, B], F32, tag="u1")
                # fold results live in their own bank as ONE accumulation
                # group (a second start would zero the sibling results).
                fold3 = fps.tile([1, 3, B], F32, tag="fold3")
                e_fold = fold3[:, 0, :]
                t_fold = fold3[:, 1, :]

                # ---------------- projection block -----------------------
                # G[v, t, b] = exp(sum_h W[v, h] enc[t, b, h] + b[v]),
                # exp fused into the PSUM->SBUF move on the Scalar engine.
                def emit_proj_block(n):
                    enc_tiles = []
                    for k in range(KH):
                        et = encp.tile([P, 512], BF16, tag=f"enc{k}",
                                       name=f"enc{k}")
                        nc.sync.dma_start(
                            et[:], encT_d.ap()[k * P:(k + 1) * P,
                                               n * 512:(n + 1) * 512])
                        enc_tiles.append(et)
                    gchunk = gexp_c[n // (NTB // NCHUNK)]
                    t0c = (n % (NTB // NCHUNK)) * 16
                    for m in range(JC):
                        ps = pps.tile([P, 512], F32, tag="pps")
                        for k in range(KH):
                            nc.tensor.matmul(ps[:], lhsT=wt_sb[:, k, m * P:(m + 1) * P],
                                             rhs=enc_tiles[k][:],
                                             start=(k == 0), stop=(k == KH - 1))
                        psv = ps[:].rearrange("p (t b) -> p t b", t=16)
                        ge = gep.tile([P, 16, B], BF16, tag="ge")
                        nc.scalar.activation(ge[:], psv,
                                             mybir.ActivationFunctionType.Exp,
                                             bias=bcol_sb[:, m:m + 1])
                        nc.sync.dma_start(
                            gchunk[m * P:(m + 1) * P, t0c:t0c + 16, :], ge[:])

                # ---------------- gather primitives ----------------------
                # row gathers of 32-wide rows, one [P,1] offset col per
                # call, landing in big [P, 64, B] buffers; the Ln + masked
                # reduction happens once at the very end.
                tv = trans_d.ap()
                trows = bass.AP(tv.tensor, tv.offset,
                                [[B, V * V // B], [1, B]])
                erows = []
                for gx in gexp_c:
                    gv = gx[:]
                    erows.append(bass.AP(gv.tensor, gv.offset,
                                         [[B, V * TCH], [1, B]]))

                def t_gather(c):
                    nc.gpsimd.indirect_dma_start(
                        out=tgall[:, c, :], out_offset=None, in_=trows,
                        in_offset=bass.IndirectOffsetOnAxis(
                            ap=toff_sb[:, c:c + 1], axis=0))

                def e_gather(c):
                    nc.gpsimd.indirect_dma_start(
                        out=egall[:, c, :], out_offset=None,
                        in_=erows[c // 16],
                        in_offset=bass.IndirectOffsetOnAxis(
                            ap=eoff_sb[:, c:c + 1], axis=0))

                # ---------------- phase C prep ---------------------------
                lviews = [g[:].rearrange("(jc p) t b -> p jc t b", p=P)
                          for g in gexp_c]
                GL = 16  # scan steps per lt load (fatter loads halve
                         # the per-step DMA-ring descriptor occupancy)
                ngroups = (steps + GL - 1) // GL

                def load_lt(grp):
                    c = (grp * GL) // TCH
                    toff0 = grp * GL - c * TCH
                    lt = ltp.tile([P, JC, GL, B], BF16, tag="lt")
                    nc.sync.dma_start(lt[:],
                                      lviews[c][:, :, toff0:toff0 + GL, :])
                    return lt

                def emit_u8(aps, t):
                    usl = u1[:]
                    for k in range(JC):
                        nc.tensor.matmul(usl, lhsT=ones_c[:], rhs=aps[k],
                                         start=(k == 0), stop=(k == JC - 1))
                    nc.vector.tensor_copy(Uall[0:1, t * B:(t + 1) * B], usl)
                    return usl

                # ---------------- prologue -------------------------------
                # wt gates the first projection matmul: load it, kick
                # block 0, then stream trans (split across both rings)
                # for the exp(transition) tiles the scan needs by t=1.
                for k in range(KH):
                    st = stp.tile([P, V], F32, tag="stg")
                    nc.sync.dma_start(st[:], wt_d.ap()[k * P:(k + 1) * P, :])
                    nc.vector.tensor_copy(wt_sb[:, k, :], st[:])
                lts = {}
                emit_proj_block(0)
                for k in range(JC):
                    st = stp.tile([P, V], F32, tag="stg")
                    nc.sync.dma_start(st[:], trans_d.ap()[k * P:(k + 1) * P, :])
                    nc.scalar.activation(ehat[:, k, :], st[:],
                                         mybir.ActivationFunctionType.Exp)
                for n in range(1, 4):
                    emit_proj_block(n)
                for g in range(3):
                    lts[g] = load_lt(g)
                if not skip_gather:
                    for c in range(64):
                        t_gather(c)
                nc.sync.dma_start(selM_sb[:], selM_d.ap()[:])
                nc.sync.dma_start(esel_sb[:], esel_d.ap()[:])
                nc.sync.dma_start(tsel_sb[:], tsel_d.ap()[:])
                nc.sync.dma_start(foldE_sb[:], foldE_d.ap()[:])
                nc.sync.dma_start(bterm_sb[:], bterm_d.ap()[:])

                # interleave remaining projection blocks into early scan
                interleave_at = {4 + 8 * j: 4 + j for j in range(NTB - 4)}

                # ---------------- phase C: the scan ----------------------
                lt = lts.pop(0)
                # t = 0: the state is just G[:, 0, :]; consume it in place.
                prev_aps = [lt[:, k, 0, :] for k in range(JC)]
                prev_uaps = prev_aps

                for t in range(1, steps):
                    sl = t % GL
                    grp = t // GL
                    if t in interleave_at:
                        emit_proj_block(interleave_at[t])
                    if sl == 0:
                        lt = lts.pop(grp)
                    if sl == 1 and grp + 3 < ngroups:
                        lts[grp + 3] = load_lt(grp + 3)

                    squads = []
                    for qi, qp in enumerate((sps0, sps1, sps2, sps3)):
                        sq = qp.tile([P, 2, B], F32, tag=f"s{qi}",
                                     name=f"s{qi}")
                        squads.append(sq)
                    a_cur = ssb.tile([P, JC, B], BF16, tag="a")
                    do_renorm = (t in renorms) and not skip_renorm

                    # phase 1: k = 0..3 for every m (ready early: the state
                    # quarters it needs fired during the previous phase 2)
                    for m in range(JC):
                        sp_m = squads[m // 2][:, m % 2, :]
                        for k in range(4):
                            nc.tensor.matmul(
                                sp_m,
                                lhsT=ehat[:, k, m * P:(m + 1) * P],
                                rhs=prev_aps[k],
                                start=(k == 0 and m % 2 == 0), stop=False)

                    # previous step's U (unscaled state); its k-th matmul
                    # only needs state quarter k//2, so it pipelines
                    usl_prev = emit_u8(prev_uaps, t - 1)
                    if do_renorm:
                        invU = smp.tile([1, B], F32, tag="invU")
                        nc.vector.reciprocal(invU[:], usl_prev)
                        bca = None
                        a_scl = ssb.tile([P, JC, B], BF16, tag="a")

                    # one emit-score gather per step once G chunks land
                    if not skip_gather and 64 <= t < 128:
                        e_gather(t - 64)

                    # phase 2: k = 4..7; after each m-pair its state quarter
                    # fires on DVE (own PSUM bank -> no read/write conflict)
                    for m in range(JC):
                        sp_m = squads[m // 2][:, m % 2, :]
                        for k in range(4, JC):
                            nc.tensor.matmul(
                                sp_m,
                                lhsT=ehat[:, k, m * P:(m + 1) * P],
                                rhs=prev_aps[k],
                                start=False,
                                stop=(k == JC - 1 and m % 2 == 1))
                        if m == 1 and do_renorm:
                            # broadcast 1/U_{t-1} once the reciprocal landed
                            bc = bps.tile([P, B], F32, tag="bc")
                            nc.tensor.matmul(bc[:], lhsT=ones_r[:],
                                             rhs=invU[:], start=True, stop=True)
                            bcap = bc[:]
                            bca = bass.AP(
                                bcap.tensor, bcap.offset,
                                [list(bcap.ap[0]), [0, 2], list(bcap.ap[1])])
                        if m % 2 == 1:
                            q = m // 2
                            nc.vector.tensor_tensor(
                                out=a_cur[:, 2 * q:2 * q + 2, :],
                                in0=squads[q][:],
                                in1=lt[:, 2 * q:2 * q + 2, sl, :],
                                op=mybir.AluOpType.mult)
                            if do_renorm:
                                nc.vector.tensor_tensor(
                                    out=a_scl[:, 2 * q:2 * q + 2, :],
                                    in0=a_cur[:, 2 * q:2 * q + 2, :],
                                    in1=bca,
                                    op=mybir.AluOpType.mult)
                    prev_uaps = [a_cur[:, k, :] for k in range(JC)]
                    if do_renorm:
                        prev_aps = [a_scl[:, k, :] for k in range(JC)]
                    else:
                        prev_aps = prev_uaps

                emit_u8(prev_uaps, steps - 1)

                # ---------------- finalize ---------------------------
                # gold-path scores: one bulk Ln over the gathered exp
                # values, then a fused masked multiply-reduce each.
                if skip_gather:
                    nc.vector.memset(t_fold, 0.0)
                    nc.vector.memset(e_fold, 0.0)
                else:
                    lnall = fin.tile([P, 64, B], F32, tag="lnall")
                    nc.scalar.activation(lnall[:], egall[:],
                                         mybir.ActivationFunctionType.Ln)
                    escr = fin.tile([P, 64, B], F32, tag="escr")
                    e_ssum = smp.tile([P, 1], F32, tag="essum")
                    nc.vector.tensor_tensor(
                        out=escr[:], in0=lnall[:], in1=esel_sb[:],
                        op=mybir.AluOpType.mult)
                    nc.vector.tensor_reduce(
                        out=e_ssum[:], in_=escr[:],
                        axis=mybir.AxisListType.XY, op=mybir.AluOpType.add)
                    nc.tensor.matmul(e_fold, lhsT=e_ssum[:], rhs=foldE_sb[:],
                                     start=True, stop=False)
                    tscr = fin.tile([P, 64, B], F32, tag="tscr")
                    t_ssum = smp.tile([P, 1], F32, tag="tssum")
                    nc.vector.tensor_tensor(
                        out=tscr[:], in0=tgall[:], in1=tsel_sb[:],
                        op=mybir.AluOpType.mult)
                    nc.vector.tensor_reduce(
                        out=t_ssum[:], in_=tscr[:],
                        axis=mybir.AxisListType.XY, op=mybir.AluOpType.add)
                    nc.tensor.matmul(t_fold, lhsT=t_ssum[:], rhs=foldE_sb[:],
                                     start=False, stop=False)

                # log Z[b] = sum_t selM[t, b] * ln U_t[b]  (mask folds the
                # t == len-1 harvest + renorm compensation, host-built).
                # Lane-shift the flat U row into [128, 64] via a DRAM
                # round-trip (linear DRAM APs; SBUF->SBUF lane moves are
                # not expressible).
                nc.sync.dma_start(uscr[:], Uall[0:1, :])
                ursh = fin.tile([P, CW], F32, tag="ursh")
                uv = uscr[:]
                nc.sync.dma_start(
                    ursh[:], bass.AP(uv.tensor, uv.offset, [[CW, P], [1, CW]]))
                # scale by 2^-48 (exact) to keep Ln inputs inside the
                # Scalar engine's valid range; compensated in host bterm
                urs2 = fin.tile([P, CW], F32, tag="urs2")
                nc.vector.tensor_scalar_mul(urs2[:], ursh[:], 2.0 ** -48)
                lnu = fin.tile([P, CW], F32, tag="lnu")
                nc.scalar.activation(lnu[:], urs2[:],
                                     mybir.ActivationFunctionType.Ln)
                msk = fin.tile([P, CW], F32, tag="msk")
                nc.vector.tensor_tensor(out=msk[:], in0=lnu[:],
                                        in1=selM_sb[:],
                                        op=mybir.AluOpType.mult)
                mred = fin.tile([P, B], F32, tag="mred")
                nc.vector.tensor_reduce(
                    out=mred[:],
                    in_=msk[:].rearrange("p (c b) -> p b c", b=B),
                    axis=mybir.AxisListType.X, op=mybir.AluOpType.add)
                zps = fold3[:, 2, :]
                nc.tensor.matmul(zps, lhsT=ones_f[:], rhs=mred[:],
                                 start=skip_gather, stop=True)
                zsb = smp.tile([1, B], F32, tag="zsb")
                nc.vector.tensor_copy(zsb[:], zps)

                d1 = smp.tile([1, B], F32, tag="d1")
                nc.vector.tensor_tensor(out=d1[:], in0=zsb[:],
                                        in1=e_fold,
                                        op=mybir.AluOpType.subtract)
                d2 = smp.tile([1, B], F32, tag="d2")
                nc.vector.tensor_tensor(out=d2[:], in0=d1[:],
                                        in1=t_fold,
                                        op=mybir.AluOpType.subtract)
                d3 = smp.tile([1, B], F32, tag="d3")
                nc.vector.tensor_tensor(out=d3[:], in0=d2[:],
                                        in1=bterm_sb[:],
                                        op=mybir.AluOpType.subtract)
                tot = smp.tile([1, 1], F32, tag="tot")
                nc.vector.tensor_reduce(out=tot[:], in_=d3[:],
                                        axis=mybir.AxisListType.X,
                                        op=mybir.AluOpType.add)
                res = smp.tile([1, 1], F32, tag="res")
                nc.vector.tensor_scalar_mul(res[:], tot[:], 1.0 / B)
                nc.sync.dma_start(loss_d.ap()[:], res[:])

    nc.compile()
    return nc


_CACHE = {}


def _get_program():
    if "nc" not in _CACHE:
        _CACHE["nc"] = _build_program()
    return _CACHE["nc"]


def _stage_inputs(enc_outs, W, b, transition, targets, lengths):
    enc_outs = np.asarray(enc_outs, dtype=np.float32)
    W = np.asarray(W, dtype=np.float32)
    b = np.asarray(b, dtype=np.float32)
    transition = np.asarray(transition, dtype=np.float32)
    targets = np.asarray(targets, dtype=np.int32)
    lengths = np.asarray(lengths, dtype=np.int32)

    encT = np.ascontiguousarray(
        enc_outs.transpose(2, 0, 1).reshape(H, TB)).astype(ml_dtypes.bfloat16)
    Wt = np.ascontiguousarray(W.T)
    bcol = np.ascontiguousarray(b.reshape(JC, P).T)

    tt = np.arange(T)[:, None]                      # [T, 1]
    mask = (tt < lengths[None, :]).astype(np.float32)        # [T, B]

    # harvest/compensation mask: selM[t, b] = 1{t == len_b - 1}
    #   + 1{t + 1 is a renorm step and t + 1 < len_b - 1}... the renorm at
    # t_r scales by 1/U_{t_r - 1}, so compensation references step t_r - 1.
    renorms = np.array(sorted(_renorm_steps()), dtype=np.int64)
    tsel = (lengths.astype(np.int64) - 1)           # [B]
    M = np.zeros((T, B), np.float32)
    np.add.at(M, (tsel, np.arange(B)), 1.0)
    M[renorms[:, None] - 1, np.arange(B)[None, :]] += (
        renorms[:, None] < tsel[None, :]).astype(np.float32)
    selM = np.ascontiguousarray(M.reshape(TB).reshape(P, TB // P))

    # gather layouts: pair q = t*B + b -> (p, c) = (q % 128, q // 128).
    # Each gather call c fetches, for every partition p, a 32-wide row of
    # the flat source; sel[p, c, :] one-hot-selects the wanted column with
    # the sequence mask folded in.
    tgrid = np.repeat(np.arange(T), B)              # [TB]
    bgrid = np.tile(np.arange(B), T)                # [TB]
    tgt_flat = targets.reshape(TB).astype(np.int64)  # targets[t, b] at q

    onehot = np.eye(B, dtype=np.float32)

    # emit: gathered from G chunk t//TCH, flat row = v*TCH + t%TCH, col b
    eoff = (tgt_flat * TCH + tgrid % TCH).astype(np.int32)
    eoff = eoff.reshape(64, P).T.copy()             # [P, 64]
    esel = (mask.reshape(TB)[:, None] * onehot[bgrid]).astype(np.float32)
    esel = np.ascontiguousarray(
        esel.reshape(64, P, B).transpose(1, 0, 2))  # [P, 64, B]

    # emit bias term (host-folded): sum_t mask * b[targets], minus the
    # compensation for the 2^-48 pre-scale of U before the device Ln
    bterm = ((mask * b[targets]).sum(axis=0)
             - 48.0 * np.log(2.0) * M.sum(axis=0)).astype(np.float32)[None, :]

    # trans: for t < T-1: trans[targets[t,b], targets[t+1,b]]
    # flat row = (tgt_t*V + tgt_t1)//32, col = tgt_t1 % 32; padded with 0
    toff = np.zeros(TB, np.int64)
    tself = np.zeros((TB, B), np.float32)
    q = tgrid < T - 1
    t_idx = tgrid[q]
    b_idx = bgrid[q]
    tgt0 = targets[t_idx, b_idx].astype(np.int64)
    tgt1 = targets[t_idx + 1, b_idx].astype(np.int64)
    toff[q] = tgt0 * (V // B) + tgt1 // B
    tself[q] = mask[t_idx + 1, b_idx][:, None] * onehot[tgt1 % B]
    toff = toff.reshape(64, P).T.astype(np.int32).copy()
    tsel = np.ascontiguousarray(tself.reshape(64, P, B).transpose(1, 0, 2))

    foldE = (np.arange(P)[:, None] % B == np.arange(B)[None, :]).astype(np.float32)

    return {
        "encT": encT, "Wt": Wt, "bcol": bcol, "trans": transition,
        "selM": selM, "eoff": eoff, "esel": esel, "toff": toff,
        "tsel": tsel, "foldE": foldE, "bterm": bterm,
    }


def kernel(enc_outs, W, b, transition, targets, lengths):
    nc = _get_program()
    in_map = _stage_inputs(enc_outs, W, b, transition, targets, lengths)
    in_maps = [in_map for _ in range(N_CORES)]
    res = run_bass_kernel_spmd(nc, in_maps, core_ids=list(range(N_CORES)))
    loss = res.results[0]["loss"]
    return np.float32(loss.reshape(())[()])


# revision 32
# speedup vs baseline: 3.6995x; 3.6995x over previous
"""CRF loss kernel for Trainium2 (8 NeuronCores).

Strategy
--------
The CRF forward scan  Z_{t+1} = logsumexp_i(Z_t[:,i] + Tr[i,j]) + logit_{t+1}
is rewritten in exp-domain as a *normalized* plain matmul recurrence:

    a_{t+1} = (a_t @ exp(Tr)) * G_{t+1},   G = exp(logits)  (precomputed)
    U_t     = sum_j a_t[b, j]                               (stored per step)

which runs on the TensorEngine: per step, 64 matmuls of
[K=128]x[M=128]x[N=32] with the exp(transition) tiles as stationary
operands (fp8, fast-weight-load) and the state a [i, b] as the moving
operand (bf16).  Every R steps the state is renormalized by 1/U_{t-1}
(one step stale, so the reciprocal/broadcast are off the critical
path).  All logarithms are LAZY: U_t is stored into a flat SBUF buffer
each step; a single end-of-kernel Ln + host-precomputed mask (encoding
the t == len[b]-1 harvest and the renorm compensation, both computable
on the host from `lengths`) reduces it to log Z[b].  The Scalar
engine's activation table holds Exp for the whole kernel (the naive
per-step Exp/Ln alternation costs two ~1.3us table reloads per step).

Per scan step the PE stream is software-pipelined:  phase-1 matmuls
(k = 0..3) of step t+1 only need early state-quarters of step t, so
they overlap the cross-engine latency of the late quarters; the state
multiply is issued as four jc-pair quarters, each in its own PSUM bank
(avoids PE-write/DVE-read bank serialization).

The projection G = exp(enc @ W.T + b) feeds the scan from DRAM in 4
t-chunks; its blocks are interleaved into the early scan steps so PE
never drains.  The gold-path score gathers exp-domain values straight
from G (and transition rows) via indirect DMA into two [128, 64, 32]
buffers; one bulk Ln + one fused multiply-reduce each at the end fold
them to per-batch scores (the bias term of the emit score is folded in
on the host).

The scan is inherently sequential over T and too small in B to shard
without per-step cross-core communication (collective latency >> step
time), so each core runs the identical full problem (data-parallel
replication costs nothing in wall time) and the host takes core 0's
scalar output.
"""
import sys
import os

sys.path.insert(0, "/opt/trn_rl_repo")

import numpy as np
import ml_dtypes

import concourse.bass as bass
import concourse.bacc as bacc
import concourse.tile as tile
from concourse import mybir
from concourse.bass_utils import run_bass_kernel_spmd

T, B, H, V = 256, 32, 512, 1024
P = 128
JC = V // P          # 8 vocab chunks
KH = H // P          # 4 hidden chunks
NTB = (T * B) // 512  # 16 tb-chunks of 512 for the projection
TB = T * B
R = 8                # renormalize the scan state every R steps
N_CORES = 8
NCHUNK = 4           # gexp DRAM tensor t-chunk split
TCH = T // NCHUNK    # 64 steps per chunk

F32 = mybir.dt.float32
BF16 = mybir.dt.bfloat16
FP8 = mybir.dt.float8e4

EHAT_DT = FP8        # dtype of exp(transition) stationary tiles
SCAN_STEPS = T       # full scan


def _renorm_steps(steps=SCAN_STEPS, r=R):
    return [t for t in range(steps) if t % r == r - 1 and t != steps - 1]


def _build_program(steps=SCAN_STEPS, ehat_dt=EHAT_DT, skip_gather=False,
                   skip_renorm=False):
    nc = bacc.Bacc("TRN2", target_bir_lowering=False, debug=False,
                   enable_asserts=False, num_devices=N_CORES)

    encT_d = nc.dram_tensor("encT", [H, TB], BF16, kind="ExternalInput")
    wt_d = nc.dram_tensor("Wt", [H, V], F32, kind="ExternalInput")
    bcol_d = nc.dram_tensor("bcol", [P, JC], F32, kind="ExternalInput")
    trans_d = nc.dram_tensor("trans", [V, V], F32, kind="ExternalInput")
    selM_d = nc.dram_tensor("selM", [P, TB // P], F32, kind="ExternalInput")
    eoff_d = nc.dram_tensor("eoff", [P, 64], mybir.dt.int32, kind="ExternalInput")
    esel_d = nc.dram_tensor("esel", [P, 64, B], F32, kind="ExternalInput")
    toff_d = nc.dram_tensor("toff", [P, 64], mybir.dt.int32, kind="ExternalInput")
    tsel_d = nc.dram_tensor("tsel", [P, 64, B], F32, kind="ExternalInput")
    foldE_d = nc.dram_tensor("foldE", [P, B], F32, kind="ExternalInput")
    bterm_d = nc.dram_tensor("bterm", [1, B], F32, kind="ExternalInput")
    loss_d = nc.dram_tensor("loss", [1, 1], F32, kind="ExternalOutput")

    renorms = set(_renorm_steps(steps))
    CW = TB // P  # 64 columns in the reshaped U / selM layout

    with tile.TileContext(nc) as tc:
        with tc.tile_pool(name="const", bufs=1) as cp, \
             tc.tile_pool(name="dram", bufs=1, space="DRAM") as dp:
            gexp_c = []
            for c in range(NCHUNK):
                gx = dp.tile([V, TCH, B], BF16, tag=f"gexp{c}", name=f"gexp{c}")
                gexp_c.append(gx)
            uscr = dp.tile([1, TB], F32, tag="uscr")

            # ---------------- phase A: constants -------------------------
            ehat = cp.tile([P, JC, V], ehat_dt, tag="ehat")
            wt_sb = cp.tile([P, KH, V], BF16, tag="wt")
            ones_c = cp.tile([P, 1], BF16, tag="ones_c")
            ones_r = cp.tile([1, P], F32, tag="ones_r")
            ones_f = cp.tile([P, 1], F32, tag="ones_f")
            bcol_sb = cp.tile([P, JC], F32, tag="bcol")
            selM_sb = cp.tile([P, CW], F32, tag="selM")
            Uall = cp.tile([1, TB], F32, tag="Uall")
            eoff_sb = cp.tile([P, 64], mybir.dt.int32, tag="eoff")
            esel_sb = cp.tile([P, 64, B], F32, tag="esel")
            toff_sb = cp.tile([P, 64], mybir.dt.int32, tag="toff")
            tsel_sb = cp.tile([P, 64, B], F32, tag="tsel")
            foldE_sb = cp.tile([P, B], F32, tag="foldE")
            bterm_sb = cp.tile([1, B], F32, tag="bterm")
            egall = cp.tile([P, 64, B], BF16, tag="egall")
            tgall = cp.tile([P, 64, B], F32, tag="tgall")

            nc.vector.memset(ones_c[:], 1.0)
            nc.vector.memset(ones_r[:], 1.0)
            nc.vector.memset(ones_f[:], 1.0)
            nc.sync.dma_start(bcol_sb[:], bcol_d.ap()[:])
            nc.sync.dma_start(toff_sb[:], toff_d.ap()[:])
            nc.sync.dma_start(eoff_sb[:], eoff_d.ap()[:])

            with tc.tile_pool(name="stage", bufs=4) as stp, \
                 tc.tile_pool(name="proj_ps", bufs=1, space="PSUM") as pps, \
                 tc.tile_pool(name="enc_p", bufs=2) as encp, \
                 tc.tile_pool(name="ge_p", bufs=3) as gep, \
                 tc.tile_pool(name="scan_sb", bufs=3) as ssb, \
                 tc.tile_pool(name="lt_p", bufs=4) as ltp, \
                 tc.tile_pool(name="s_ps0", bufs=1, space="PSUM") as sps0, \
                 tc.tile_pool(name="s_ps1", bufs=1, space="PSUM") as sps1, \
                 tc.tile_pool(name="s_ps2", bufs=1, space="PSUM") as sps2, \
                 tc.tile_pool(name="s_ps3", bufs=1, space="PSUM") as sps3, \
                 tc.tile_pool(name="u_ps", bufs=1, space="PSUM") as ups, \
                 tc.tile_pool(name="fold_ps", bufs=1, space="PSUM") as fps, \
                 tc.tile_pool(name="bc_ps", bufs=1, space="PSUM") as bps, \
                 tc.tile_pool(name="sm", bufs=4) as smp, \
                 tc.tile_pool(name="fin", bufs=1) as fin:

                # single-slot U bank: rewriting the same [1, B] slice each
                # step makes the ACT copy a tracked WAR predecessor of the
                # next step's start (which zeroes the whole 2KB region).
                u1 = ups.tile([1, B], F32, tag="u1")
                # fold results live in their own bank as ONE accumulation
                # group (a second start would zero the sibling results).
                fold3 = fps.tile([1, 3, B], F32, tag="fold3")
                e_fold = fold3[:, 0, :]
                t_fold = fold3[:, 1, :]

                # ---------------- projection block -----------------------
                # G[v, t, b] = exp(sum_h W[v, h] enc[t, b, h] + b[v]),
                # exp fused into the PSUM->SBUF move on the Scalar engine.
                def emit_proj_block(n):
                    enc_tiles = []
                    for k in range(KH):
                        et = encp.tile([P, 512], BF16, tag=f"enc{k}",
                                       name=f"enc{k}")
                        nc.sync.dma_start(
                            et[:], encT_d.ap()[k * P:(k + 1) * P,
                                               n * 512:(n + 1) * 512])
                        enc_tiles.append(et)
                    gchunk = gexp_c[n // (NTB // NCHUNK)]
                    t0c = (n % (NTB // NCHUNK)) * 16
                    for m in range(JC):
                        ps = pps.tile([P, 512], F32, tag="pps")
                        for k in range(KH):
                            nc.tensor.matmul(ps[:], lhsT=wt_sb[:, k, m * P:(m + 1) * P],
                                             rhs=enc_tiles[k][:],
                                             start=(k == 0), stop=(k == KH - 1))
                        psv = ps[:].rearrange("p (t b) -> p t b", t=16)
                        ge = gep.tile([P, 16, B], BF16, tag="ge")
                        nc.scalar.activation(ge[:], psv,
                                             mybir.ActivationFunctionType.Exp,
                                             bias=bcol_sb[:, m:m + 1])
                        nc.sync.dma_start(
                            gchunk[m * P:(m + 1) * P, t0c:t0c + 16, :], ge[:])

                # ---------------- gather primitives ----------------------
                # row gathers of 32-wide rows, one [P,1] offset col per
                # call, landing in big [P, 64, B] buffers; the Ln + masked
                # reduction happens once at the very end.
                tv = trans_d.ap()
                trows = bass.AP(tv.tensor, tv.offset,
                                [[B, V * V // B], [1, B]])
                erows = []
                for gx in gexp_c:
                    gv = gx[:]
                    erows.append(bass.AP(gv.tensor, gv.offset,
                                         [[B, V * TCH], [1, B]]))

                def t_gather(c):
                    nc.gpsimd.indirect_dma_start(
                        out=tgall[:, c, :], out_offset=None, in_=trows,
                        in_offset=bass.IndirectOffsetOnAxis(
                            ap=toff_sb[:, c:c + 1], axis=0))

                def e_gather(c):
                    nc.gpsimd.indirect_dma_start(
                        out=egall[:, c, :], out_offset=None,
                        in_=erows[c // 16],
                        in_offset=bass.IndirectOffsetOnAxis(
                            ap=eoff_sb[:, c:c + 1], axis=0))

                # ---------------- phase C prep ---------------------------
                lviews = [g[:].rearrange("(jc p) t b -> p jc t b", p=P)
                          for g in gexp_c]
                ngroups = (steps + 7) // 8

                def load_lt(grp):
                    c = (grp * 8) // TCH
                    toff0 = grp * 8 - c * TCH
                    lt = ltp.tile([P, JC, 8, B], BF16, tag="lt")
                    # alternate rings: halves the per-ring occupancy of the
                    # scan's dominant DMA stream (same 8-step descriptor
                    # pattern -- group size is a lowering sweet spot)
                    ring = nc.sync if grp % 2 == 0 else nc.gpsimd
                    ring.dma_start(lt[:],
                                   lviews[c][:, :, toff0:toff0 + 8, :])
                    return lt

                def emit_u8(aps, t):
                    usl = u1[:]
                    for k in range(JC):
                        nc.tensor.matmul(usl, lhsT=ones_c[:], rhs=aps[k],
                                         start=(k == 0), stop=(k == JC - 1))
                    nc.vector.tensor_copy(Uall[0:1, t * B:(t + 1) * B], usl)
                    return usl

                # ---------------- prologue -------------------------------
                # wt gates the first projection matmul: load it, kick
                # block 0, then stream trans (split across both rings)
                # for the exp(transition) tiles the scan needs by t=1.
                for k in range(KH):
                    st = stp.tile([P, V], F32, tag="stg")
                    nc.sync.dma_start(st[:], wt_d.ap()[k * P:(k + 1) * P, :])
                    nc.vector.tensor_copy(wt_sb[:, k, :], st[:])
                lts = {}
                emit_proj_block(0)
                for k in range(JC):
                    st = stp.tile([P, V], F32, tag="stg")
                    nc.sync.dma_start(st[:], trans_d.ap()[k * P:(k + 1) * P, :])
                    nc.scalar.activation(ehat[:, k, :], st[:],
                                         mybir.ActivationFunctionType.Exp)
                for n in range(1, 4):
                    emit_proj_block(n)
                for g in range(4):
                    lts[g] = load_lt(g)
                if not skip_gather:
                    for c in range(64):
                        t_gather(c)
                nc.sync.dma_start(selM_sb[:], selM_d.ap()[:])
                nc.sync.dma_start(esel_sb[:], esel_d.ap()[:])
                nc.sync.dma_start(tsel_sb[:], tsel_d.ap()[:])
                nc.sync.dma_start(foldE_sb[:], foldE_d.ap()[:])
                nc.sync.dma_start(bterm_sb[:], bterm_d.ap()[:])

                # interleave remaining projection blocks into early scan
                interleave_at = {4 + 8 * j: 4 + j for j in range(NTB - 4)}

                # ---------------- phase C: the scan ----------------------
                lt = lts.pop(0)
                # t = 0: the state is just G[:, 0, :]; consume it in place.
                prev_aps = [lt[:, k, 0, :] for k in range(JC)]
                prev_uaps = prev_aps

                for t in range(1, steps):
                    sl = t % 8
                    grp = t // 8
                    if t in interleave_at:
                        emit_proj_block(interleave_at[t])
                    if sl == 0:
                        lt = lts.pop(grp)
                    if sl == 1 and grp + 4 < ngroups:
                        lts[grp + 4] = load_lt(grp + 4)

                    squads = []
                    for qi, qp in enumerate((sps0, sps1, sps2, sps3)):
                        sq = qp.tile([P, 2, B], F32, tag=f"s{qi}",
                                     name=f"s{qi}")
                        squads.append(sq)
                    a_cur = ssb.tile([P, JC, B], BF16, tag="a")
                    do_renorm = (t in renorms) and not skip_renorm

                    # phase 1: k = 0..3 for every m (ready early: the state
                    # quarters it needs fired during the previous phase 2)
                    for m in range(JC):
                        sp_m = squads[m // 2][:, m % 2, :]
                        for k in range(4):
                            nc.tensor.matmul(
                                sp_m,
                                lhsT=ehat[:, k, m * P:(m + 1) * P],
                                rhs=prev_aps[k],
                                start=(k == 0 and m % 2 == 0), stop=False)

                    # previous step's U (unscaled state); its k-th matmul
                    # only needs state quarter k//2, so it pipelines
                    usl_prev = emit_u8(prev_uaps, t - 1)
                    if do_renorm:
                        invU = smp.tile([1, B], F32, tag="invU")
                        nc.vector.reciprocal(invU[:], usl_prev)
                        bca = None
                        a_scl = ssb.tile([P, JC, B], BF16, tag="a")

                    # one emit-score gather per step once G chunks land
                    if not skip_gather and 64 <= t < 128:
                        e_gather(t - 64)

                    # phase 2: k = 4..7; after each m-pair its state quarter
                    # fires on DVE (own PSUM bank -> no read/write conflict)
                    for m in range(JC):
                        sp_m = squads[m // 2][:, m % 2, :]
                        for k in range(4, JC):
                            nc.tensor.matmul(
                                sp_m,
                                lhsT=ehat[:, k, m * P:(m + 1) * P],
                                rhs=prev_aps[k],
                                start=False,
                                stop=(k == JC - 1 and m % 2 == 1))
                        if m == 1 and do_renorm:
                            # broadcast 1/U_{t-1} once the reciprocal landed
                            bc = bps.tile([P, B], F32, tag="bc")
                            nc.tensor.matmul(bc[:], lhsT=ones_r[:],
                                             rhs=invU[:], start=True, stop=True)
                            bcap = bc[:]
                            bca = bass.AP(
                                bcap.tensor, bcap.offset,
                                [list(bcap.ap[0]), [0, 2], list(bcap.ap[1])])
                        if m % 2 == 1:
                            q = m // 2
                            nc.vector.tensor_tensor(
                                out=a_cur[:, 2 * q:2 * q + 2, :],
                                in0=squads[q][:],
                                in1=lt[:, 2 * q:2 * q + 2, sl, :],
                                op=mybir.AluOpType.mult)
                            if do_renorm:
                                nc.vector.tensor_tensor(
                                    out=a_scl[:, 2 * q:2 * q + 2, :],
                                    in0=a_cur[:, 2 * q:2 * q + 2, :],
                                    in1=bca,
                                    op=mybir.AluOpType.mult)
                    prev_uaps = [a_cur[:, k, :] for k in range(JC)]
                    if do_renorm:
                        prev_aps = [a_scl[:, k, :] for k in range(JC)]
                    else:
                        prev_aps = prev_uaps

                emit_u8(prev_uaps, steps - 1)

                # ---------------- finalize ---------------------------
                # gold-path scores: one bulk Ln over the gathered exp
                # values, then a fused masked multiply-reduce each.
                if skip_gather:
                    nc.vector.memset(t_fold, 0.0)
                    nc.vector.memset(e_fold, 0.0)
                else:
                    lnall = fin.tile([P, 64, B], F32, tag="lnall")
                    nc.scalar.activation(lnall[:], egall[:],
                                         mybir.ActivationFunctionType.Ln)
                    escr = fin.tile([P, 64, B], F32, tag="escr")
                    e_ssum = smp.tile([P, 1], F32, tag="essum")
                    nc.vector.tensor_tensor(
                        out=escr[:], in0=lnall[:], in1=esel_sb[:],
                        op=mybir.AluOpType.mult)
                    nc.vector.tensor_reduce(
                        out=e_ssum[:], in_=escr[:],
                        axis=mybir.AxisListType.XY, op=mybir.AluOpType.add)
                    nc.tensor.matmul(e_fold, lhsT=e_ssum[:], rhs=foldE_sb[:],
                                     start=True, stop=False)
                    tscr = fin.tile([P, 64, B], F32, tag="tscr")
                    t_ssum = smp.tile([P, 1], F32, tag="tssum")
                    nc.vector.tensor_tensor(
                        out=tscr[:], in0=tgall[:], in1=tsel_sb[:],
                        op=mybir.AluOpType.mult)
                    nc.vector.tensor_reduce(
                        out=t_ssum[:], in_=tscr[:],
                        axis=mybir.AxisListType.XY, op=mybir.AluOpType.add)
                    nc.tensor.matmul(t_fold, lhsT=t_ssum[:], rhs=foldE_sb[:],
                                     start=False, stop=False)

                # log Z[b] = sum_t selM[t, b] * ln U_t[b]  (mask folds the
                # t == len-1 harvest + renorm compensation, host-built).
                # Lane-shift the flat U row into [128, 64] via a DRAM
                # round-trip (linear DRAM APs; SBUF->SBUF lane moves are
                # not expressible).
                nc.sync.dma_start(uscr[:], Uall[0:1, :])
                ursh = fin.tile([P, CW], F32, tag="ursh")
                uv = uscr[:]
                nc.sync.dma_start(
                    ursh[:], bass.AP(uv.tensor, uv.offset, [[CW, P], [1, CW]]))
                # scale by 2^-48 (exact) to keep Ln inputs inside the
                # Scalar engine's valid range; compensated in host bterm
                urs2 = fin.tile([P, CW], F32, tag="urs2")
                nc.vector.tensor_scalar_mul(urs2[:], ursh[:], 2.0 ** -48)
                lnu = fin.tile([P, CW], F32, tag="lnu")
                nc.scalar.activation(lnu[:], urs2[:],
                                     mybir.ActivationFunctionType.Ln)
                msk = fin.tile([P, CW], F32, tag="msk")
                nc.vector.tensor_tensor(out=msk[:], in0=lnu[:],
                                        in1=selM_sb[:],
                                        op=mybir.AluOpType.mult)
                mred = fin.tile([P, B], F32, tag="mred")
                nc.vector.tensor_reduce(
                    out=mred[:],
                    in_=msk[:].rearrange("p (c b) -> p b c", b=B),
                    axis=mybir.AxisListType.X, op=mybir.AluOpType.add)
                zps = fold3[:, 2, :]
                nc.tensor.matmul(zps, lhsT=ones_f[:], rhs=mred[:],
                                 start=skip_gather, stop=True)
                zsb = smp.tile([1, B], F32, tag="zsb")
                nc.vector.tensor_copy(zsb[:], zps)

                d1 = smp.tile([1, B], F32, tag="d1")
                nc.vector.tensor_tensor(out=d1[:], in0=zsb[:],
                                        in1=e_fold,
                                        op=mybir.AluOpType.subtract)
                d2 = smp.tile([1, B], F32, tag="d2")
                nc.vector.tensor_tensor(out=d2[:], in0=d1[:],
                                        in1=t_fold,
                                        op=mybir.AluOpType.subtract)
                d3 = smp.tile([1, B], F32, tag="d3")
                nc.vector.tensor_tensor(out=d3[:], in0=d2[:],
                                        in1=bterm_sb[:],
                                        op=mybir.AluOpType.subtract)
                tot = smp.tile([1, 1], F32, tag="tot")
                nc.vector.tensor_reduce(out=tot[:], in_=d3[:],
                                        axis=mybir.AxisListType.X,
                                        op=mybir.AluOpType.add)
                res = smp.tile([1, 1], F32, tag="res")
                nc.vector.tensor_scalar_mul(res[:], tot[:], 1.0 / B)
                nc.sync.dma_start(loss_d.ap()[:], res[:])

    nc.compile()
    return nc


_CACHE = {}


def _get_program():
    if "nc" not in _CACHE:
        _CACHE["nc"] = _build_program()
    return _CACHE["nc"]


def _stage_inputs(enc_outs, W, b, transition, targets, lengths):
    enc_outs = np.asarray(enc_outs, dtype=np.float32)
    W = np.asarray(W, dtype=np.float32)
    b = np.asarray(b, dtype=np.float32)
    transition = np.asarray(transition, dtype=np.float32)
    targets = np.asarray(targets, dtype=np.int32)
    lengths = np.asarray(lengths, dtype=np.int32)

    encT = np.ascontiguousarray(
        enc_outs.transpose(2, 0, 1).reshape(H, TB)).astype(ml_dtypes.bfloat16)
    Wt = np.ascontiguousarray(W.T)
    bcol = np.ascontiguousarray(b.reshape(JC, P).T)

    tt = np.arange(T)[:, None]                      # [T, 1]
    mask = (tt < lengths[None, :]).astype(np.float32)        # [T, B]

    # harvest/compensation mask: selM[t, b] = 1{t == len_b - 1}
    #   + 1{t + 1 is a renorm step and t + 1 < len_b - 1}... the renorm at
    # t_r scales by 1/U_{t_r - 1}, so compensation references step t_r - 1.
    renorms = np.array(sorted(_renorm_steps()), dtype=np.int64)
    tsel = (lengths.astype(np.int64) - 1)           # [B]
    M = np.zeros((T, B), np.float32)
    np.add.at(M, (tsel, np.arange(B)), 1.0)
    M[renorms[:, None] - 1, np.arange(B)[None, :]] += (
        renorms[:, None] < tsel[None, :]).astype(np.float32)
    selM = np.ascontiguousarray(M.reshape(TB).reshape(P, TB // P))

    # gather layouts: pair q = t*B + b -> (p, c) = (q % 128, q // 128).
    # Each gather call c fetches, for every partition p, a 32-wide row of
    # the flat source; sel[p, c, :] one-hot-selects the wanted column with
    # the sequence mask folded in.
    tgrid = np.repeat(np.arange(T), B)              # [TB]
    bgrid = np.tile(np.arange(B), T)                # [TB]
    tgt_flat = targets.reshape(TB).astype(np.int64)  # targets[t, b] at q

    onehot = np.eye(B, dtype=np.float32)

    # emit: gathered from G chunk t//TCH, flat row = v*TCH + t%TCH, col b
    eoff = (tgt_flat * TCH + tgrid % TCH).astype(np.int32)
    eoff = eoff.reshape(64, P).T.copy()             # [P, 64]
    esel = (mask.reshape(TB)[:, None] * onehot[bgrid]).astype(np.float32)
    esel = np.ascontiguousarray(
        esel.reshape(64, P, B).transpose(1, 0, 2))  # [P, 64, B]

    # emit bias term (host-folded): sum_t mask * b[targets], minus the
    # compensation for the 2^-48 pre-scale of U before the device Ln
    bterm = ((mask * b[targets]).sum(axis=0)
             - 48.0 * np.log(2.0) * M.sum(axis=0)).astype(np.float32)[None, :]

    # trans: for t < T-1: trans[targets[t,b], targets[t+1,b]]
    # flat row = (tgt_t*V + tgt_t1)//32, col = tgt_t1 % 32; padded with 0
    toff = np.zeros(TB, np.int64)
    tself = np.zeros((TB, B), np.float32)
    q = tgrid < T - 1
    t_idx = tgrid[q]
    b_idx = bgrid[q]
    tgt0 = targets[t_idx, b_idx].astype(np.int64)
    tgt1 = targets[t_idx + 1, b_idx].astype(np.int64)
    toff[q] = tgt0 * (V // B) + tgt1 // B
    tself[q] = mask[t_idx + 1, b_idx][:, None] * onehot[tgt1 % B]
    toff = toff.reshape(64, P).T.astype(np.int32).copy()
    tsel = np.ascontiguousarray(tself.reshape(64, P, B).transpose(1, 0, 2))

    foldE = (np.arange(P)[:, None] % B == np.arange(B)[None, :]).astype(np.float32)

    return {
        "encT": encT, "Wt": Wt, "bcol": bcol, "trans": transition,
        "selM": selM, "eoff": eoff, "esel": esel, "toff": toff,
        "tsel": tsel, "foldE": foldE, "bterm": bterm,
    }


def kernel(enc_outs, W, b, transition, targets, lengths):
    nc = _get_program()
    in_map = _stage_inputs(enc_outs, W, b, transition, targets, lengths)
    in_maps = [in_map for _ in range(N_CORES)]
    res = run_bass_kernel_spmd(nc, in_maps, core_ids=list(range(N_CORES)))
    loss = res.results[0]["loss"]
    return np.float32(loss.reshape(())[()])
